# revision 31
# baseline (speedup 1.0000x reference)
"""Multi-head cross-attention kernel for Trainium2, 8 NeuronCores.

Reference computation (B=2, S=2048, D=1024, H=16, hd=64):
    kv = x @ Wkv + bkv ; q = y @ Wq + bq
    per head: s = q k^T / 8 (+ mask, all-zero per spec), a = softmax(s)
    out = concat_h(a v) @ Wo + bo

Sharding: batch (2-way) x head-groups (4 heads/core).  Cores 0-3 own batch 0,
cores 4-7 own batch 1; within a batch group, core j owns heads 4j..4j+3 and,
after an AllToAll of normalized per-head attention outputs, computes the
output projection for four disjoint 64-row sq slices (one per 512-sq block).

Host-side shard prep:
  - x[b], y[b] transposed to [D, S] fp16 so the contraction dim lands on
    SBUF partitions.
  - Wq / Wk / Wv sliced per head group and packed into one [D, 768] tensor;
    k-bias dropped (softmax shift-invariant), v-bias folded into an
    effective output bias bo_eff = bv @ Wo + bo.
  - mask is all-zeros per the problem spec -> additive zero, skipped.

Device dataflow per core (its batch b, heads hg..hg+3), all matmuls fp16
with fp32 PSUM accumulation:
  kT[pair, S], qT[pair, S] = W^T xT / W^T yT;  v[S, 4x(64|ones)]
  per (sq-block 512, head pair): per sk-chunk 128:
    scoresT = row-packed K=64 matmul pair -> PSUM
    exp: head0 on ACT (exp, scale=1/8); head1 on DVE via Schraudolph
      bit-trick (i16 = s*(1477.32/8) + 15315.5, bitcast i16->f16 ~= 2^t,
      max rel err ~3%) -- splits softmax-exp across two engines so the
      PE stream, not ACT, paces the attention phase.
    PV accumulate with lhsT=[v|1] (M=65): unnormalized vals + denominator.
  normalize (DMA-free: engines only, so a straggler rank's pending
    collective -- whose descriptors gate shared DMA rings -- can never
    stall this chain or, transitively, the PE): d row -> fp16 SBUF row,
    K=1 matmul broadcasts it down 64 PSUM partitions,
    reciprocal_approx_fast off PSUM, one fused scalar_tensor_tensor
    (PSUM * rep -> fresh per-(blk,h) fp16 nv tile).
  Per-block 8-rank AllToAll of normalized valsT (fp16). vf tiles for the
  output projection are prefetched mid-attention on the idle sync queue;
  all four outprojs run after attn3 (only the last ~4.5us waits on the
  final collective). A small full-array matmul every 4th chunk keeps the
  HAM activity monitor from re-throttling the PE clock (attention
  matmuls alone use half the array and read as "idle").
  outproj: vf tiles pack both batches' rows (M=128), out = vf^T Wo + bo.
"""

import numpy as np

import concourse.bass as bass
import concourse.bacc as bacc
import concourse.mybir as mybir
from concourse.tile import TileContext
from concourse.bass_utils import run_bass_kernel_spmd

B, S, D = 2, 2048, 1024
H, HD = 16, 64
N_CORES = 8
GROUP = 4              # cores per batch group
HPC = H // GROUP       # heads per core (4)
NV = HPC * HD          # local vals rows (256)
NBLK = 4               # sq blocks
SQB = S // NBLK        # 512
PIECE = SQB // N_CORES  # 64: sq rows delivered to each rank per AllToAll
NKC = S // 128         # 16 sk chunks
NDC = D // 128         # 8 contraction chunks

F32 = mybir.dt.float32
FP16 = mybir.dt.float16
I16 = mybir.dt.int16
EXP = mybir.ActivationFunctionType.Exp
MULT = mybir.AluOpType.mult
ADD = mybir.AluOpType.add

SCALE = 0.125          # 1/sqrt(HD)
A16 = float(1024.0 / np.log(2.0)) * SCALE   # Schraudolph mult (scale folded)
B16 = float(1024 * 15 - 44.5)               # Schraudolph bias


def build_kernel():
    nc = bacc.Bacc("TRN2", target_bir_lowering=False, debug=False,
                   num_devices=N_CORES)

    yT = nc.declare_dram_parameter("yT", [D, S], FP16, isOutput=False)
    xT = nc.declare_dram_parameter("xT", [D, S], FP16, isOutput=False)
    # packed [q | k | v] head-group weight slices
    wqkv = nc.declare_dram_parameter("wqkv", [D, 3 * NV], FP16, isOutput=False)
    wo = nc.declare_dram_parameter("wo", [D, D], FP16, isOutput=False)
    bq = nc.declare_dram_parameter("bq", [NV], F32, isOutput=False)
    bo = nc.declare_dram_parameter("bo", [D], FP16, isOutput=False)
    # out rows: (blk, batch, 64 sq) for this rank's sq windows
    out = nc.declare_dram_parameter("out", [NBLK * B * PIECE, D], F32,
                                    isOutput=True)

    # 8-rank AllToAll: shard j = my heads' vals for rank j's sq window.
    cc_in = [nc.dram_tensor(f"cc_in{b}", [N_CORES, NV, PIECE], FP16)
             for b in range(NBLK)]
    cc_out = [nc.dram_tensor(f"cc_out{b}", [N_CORES * NV, PIECE], FP16)
              for b in range(NBLK)]
    groups = [[0, 1, 2, 3, 4, 5, 6, 7]]

    with TileContext(nc) as tc:
        with (
            tc.tile_pool(name="acts", bufs=1) as acts,        # persistent
            tc.tile_pool(name="stream", bufs=2) as stream,
            tc.tile_pool(name="attn", bufs=3) as attn,
            tc.tile_pool(name="psum", bufs=1, space="PSUM") as psum,
        ):
            # ---- persistent tiles ----
            xT_sb = [acts.tile([128, S], FP16, tag=f"xT{i}", name=f"xT{i}")
                     for i in range(NDC)]
            yT_sb = [acts.tile([128, S], FP16, tag=f"yT{i}", name=f"yT{i}")
                     for i in range(NDC)]
            wqkv_sb = [acts.tile([128, 3 * NV], FP16, tag=f"wqkv{i}",
                                 name=f"wqkv{i}") for i in range(NDC)]
            wo_sb = [acts.tile([128, D], FP16, tag=f"wo{i}", name=f"wo{i}")
                     for i in range(NDC)]
            qT_sb = [acts.tile([128, S], FP16, tag=f"qT{i}", name=f"qT{i}")
                     for i in range(2)]
            kT_sb = [acts.tile([128, S], FP16, tag=f"kT{i}", name=f"kT{i}")
                     for i in range(2)]
            v_sb = [acts.tile([128, HPC * (HD + 1)], FP16, tag=f"v{i}",
                              name=f"v{i}") for i in range(NKC)]
            ones_col = acts.tile([HD + 1, HD], FP16, tag="ones_col")
            bq_sb = acts.tile([128, 2], F32, tag="bq")
            bo_bc = acts.tile([128, D], FP16, tag="bo")

            nc.vector.memset(ones_col[:], 1.0)
            # load priority: weights for k/v proj, then x, then y, wo last.
            # dma_start costs ~600ns of sequencer issue time, so the 16
            # first-matmul-critical loads are split across both HWDGE
            # issuers (wqkv on ACT, xT on SP) to halve the preamble
            for i in range(NDC):
                nc.scalar.dma_start(out=wqkv_sb[i][:],
                                    in_=wqkv[128 * i:128 * (i + 1), :])
                nc.sync.dma_start(
                    out=xT_sb[i][:, 0:512],
                    in_=xT[128 * i:128 * (i + 1), 0:512])
            nc.scalar.dma_start(out=bq_sb[:],
                                in_=bq.rearrange("(c p) -> p c", p=128))
            nc.scalar.dma_start(out=bo_bc[:],
                                in_=bo[None, :].to_broadcast((128, D)))
            for sb in range(1, 4):
                for i in range(NDC):
                    nc.sync.dma_start(
                        out=xT_sb[i][:, 512 * sb:512 * (sb + 1)],
                        in_=xT[128 * i:128 * (i + 1), 512 * sb:512 * (sb + 1)])
            for sb in range(4):
                for i in range(NDC):
                    nc.sync.dma_start(
                        out=yT_sb[i][:, 512 * sb:512 * (sb + 1)],
                        in_=yT[128 * i:128 * (i + 1), 512 * sb:512 * (sb + 1)])
            for i in range(NDC):
                nc.sync.dma_start(out=wo_sb[i][:],
                                  in_=wo[128 * i:128 * (i + 1), :])

            # ---- projections ----
            # kT from xT
            for sb in range(4):
                for cc in range(2):
                    ps_k = psum.tile([128, 512], F32, tag="sc", name="ps_k",
                                     bufs=3)
                    for i in range(NDC):
                        nc.tensor.matmul(
                            ps_k[:],
                            wqkv_sb[i][:, NV + 128 * cc:NV + 128 * (cc + 1)],
                            xT_sb[i][:, 512 * sb:512 * (sb + 1)],
                            start=(i == 0), stop=(i == NDC - 1))
                    nc.vector.tensor_copy(
                        kT_sb[cc][:, 512 * sb:512 * (sb + 1)], ps_k[:])
            # v from xT
            for ks in range(NKC):
                ps_v = psum.tile([128, NV], F32, tag="pv", name="ps_v", bufs=4)
                for i in range(NDC):
                    nc.tensor.matmul(
                        ps_v[:], xT_sb[i][:, 128 * ks:128 * (ks + 1)],
                        wqkv_sb[i][:, 2 * NV:3 * NV],
                        start=(i == 0), stop=(i == NDC - 1))
                nc.vector.memset(v_sb[ks][:], 1.0)
                nc.vector.tensor_copy(
                    v_sb[ks][:].rearrange("p (h c) -> p h c",
                                          c=HD + 1)[:, :, 0:HD],
                    ps_v[:].rearrange("p (h c) -> p h c", c=HD))
            # qT from yT (with bias)
            for sb in range(4):
                for cc in range(2):
                    ps_q = psum.tile([128, 512], F32, tag="sc", name="ps_q",
                                     bufs=3)
                    for i in range(NDC):
                        nc.tensor.matmul(
                            ps_q[:],
                            wqkv_sb[i][:, 128 * cc:128 * (cc + 1)],
                            yT_sb[i][:, 512 * sb:512 * (sb + 1)],
                            start=(i == 0), stop=(i == NDC - 1))
                    nc.vector.tensor_scalar_add(
                        qT_sb[cc][:, 512 * sb:512 * (sb + 1)], ps_q[:],
                        bq_sb[:, cc:cc + 1])

            # ---- attention + pipelined outproj ----
            def normalize_pair(blk, pair, pv_ps):
                # DMA-free normalize: engines only, so a straggler rank's
                # pending collective (which gates descriptors in the shared
                # DMA rings) can never stall this chain or the PE stream
                sq0 = SQB * blk
                for hh in range(2):
                    h = 2 * pair + hh
                    # d row (partition 64) -> SBUF fp16 row, then a K=1
                    # matmul broadcasts it down 64 PSUM partitions
                    dstage = stream.tile([HD + 1, SQB], FP16, tag="dstage")
                    nc.scalar.copy(dstage[HD:HD + 1, :],
                                   pv_ps[hh][HD:HD + 1, :])
                    rep_ps = psum.tile([128, 512], F32, tag="op",
                                       name="rep_ps", bufs=1)
                    nc.tensor.matmul(rep_ps[0:HD, :],
                                     ones_col[HD:HD + 1, :],
                                     dstage[HD:HD + 1, :],
                                     tile_position=(64, 0))
                    rep = stream.tile([HD, SQB], F32, tag="rep")
                    nc.vector.reciprocal_approx_fast(rep[:], rep_ps[0:HD, :])
                    # fresh per-(blk,h) nv tile: a pending collective's
                    # cc_in read can then never WAR-block the next block's
                    # normalize (and transitively the PE)
                    nv = stream.tile([HD, SQB], FP16, tag=f"nv{h}",
                                     name=f"nv{h}", bufs=2)
                    nc.vector.scalar_tensor_tensor(
                        nv[:], pv_ps[hh][0:HD, :], 1.0, rep[:],
                        mybir.AluOpType.bypass, MULT)
                    nc.scalar.dma_start(
                        out=cc_in[blk][:, HD * h:HD * (h + 1), :]
                        .rearrange("g p q -> p g q"),
                        in_=nv[:])

            def attn_block(blk, mid_hook=None):
                # Software-pipelined chunk stream, flattened across the two
                # head pairs: scores(k+1) is emitted before PV(k) so the PE
                # never waits on exp; a tiny full-array matmul per chunk
                # keeps the HAM activity monitor from re-throttling the PE
                # clock (attention matmuls alone use half the array).
                sq0 = SQB * blk
                pv_tiles = {}
                inflight = []

                def emit_pv(pair, sc):
                    for hh in range(2):
                        h = 2 * pair + hh
                        nc.tensor.matmul(
                            pv_tiles[pair][hh][:],
                            v_sb[sc][:, (HD + 1) * h:(HD + 1) * (h + 1)],
                            at_tiles[(pair, sc)][hh][:],
                            start=(sc == 0), stop=(sc == NKC - 1))

                at_tiles = {}
                seq = [(p, s) for p in range(2) for s in range(NKC)]
                # 2-chunk groups: both score pairs back-to-back, then the
                # four PV matmuls back-to-back -- geometry transitions
                # (K=64 pair <-> K=128) expose ~200ns of drain+LDW each,
                # so batching same-geometry matmuls halves that cost
                for g in range(0, len(seq), 2):
                    for k in (g, g + 1):
                        pair, sc = seq[k]
                        if sc == 0:
                            pv_tiles[pair] = [
                                psum.tile([128, SQB], F32, tag="pv",
                                          name=f"pv{hh}", bufs=4)[:HD + 1, :]
                                for hh in range(2)]
                        sc_ps = [psum.tile([128, SQB], F32, tag="sc",
                                           name=f"sc{hh}", bufs=3)
                                 for hh in range(2)]
                        at_sb = [attn.tile([128, SQB], FP16, tag=f"at{hh}",
                                           name=f"at{hh}", bufs=4)
                                 for hh in range(2)]
                        at_tiles[(pair, sc)] = at_sb
                        for hh in range(2):  # row-packed head pair
                            nc.tensor.matmul(
                                sc_ps[hh][:],
                                kT_sb[pair][64 * hh:64 * (hh + 1),
                                            128 * sc:128 * (sc + 1)],
                                qT_sb[pair][64 * hh:64 * (hh + 1),
                                            sq0:sq0 + SQB],
                                tile_position=(64 * hh, 0))
                        # exp split across engines: ACT for head0, DVE
                        # Schraudolph for head1
                        nc.scalar.activation(at_sb[0][:], sc_ps[0][:], EXP,
                                             scale=SCALE)
                        nc.vector.tensor_scalar(
                            at_sb[1][:].bitcast(I16), sc_ps[1][:],
                            A16, B16, MULT, ADD)
                        inflight.append((pair, sc))
                    if g % 8 == 0:
                        # HAM warm-keeper: full-array K=128 M=128, N=32
                        warm_ps = psum.tile([128, 512], F32, tag="op",
                                            name="warm_ps", bufs=1)
                        nc.tensor.matmul(warm_ps[:, 0:32],
                                         wqkv_sb[0][:, 0:128],
                                         xT_sb[0][:, 0:32])
                    while len(inflight) > 2:
                        done = inflight.pop(0)
                        emit_pv(*done)
                        if done[1] == NKC - 1:
                            normalize_pair(blk, done[0], pv_tiles[done[0]])
                        del at_tiles[done]
                    if g == NKC and mid_hook is not None:
                        mid_hook()
                for done in inflight:
                    emit_pv(*done)
                    if done[1] == NKC - 1:
                        normalize_pair(blk, done[0], pv_tiles[done[0]])
                    del at_tiles[done]
                nc.gpsimd.collective_compute(
                    "AllToAll", mybir.AluOpType.bypass,
                    ins=[cc_in[blk][:]], outs=[cc_out[blk][:]],
                    replica_groups=groups)

            vf_tiles = {}

            def outproj_load(blk, split=False):
                # vf tiles pack both batches: cols 0-63 batch0, 64-127
                # batch1 -- one 3D-AP DMA per tile, and (for the last,
                # tail-critical block) issue split across both HWDGE
                # queues to halve sequencer issue time
                vf = [stream.tile([128, 128], FP16, tag=f"vf{i}",
                                  name=f"vf{i}", bufs=4) for i in range(NDC)]
                vf_tiles[blk] = vf
                src3 = cc_out[blk].rearrange("(j p) q -> p j q", j=2)
                for i in range(NDC):
                    eng = nc.scalar if (split and i % 2) else nc.sync
                    eng.dma_start(
                        out=vf[i][:].rearrange("p (j q) -> p j q", j=2),
                        in_=src3[128 * i:128 * (i + 1)])

            def outproj_block(blk):
                vf = vf_tiles[blk]
                for dcb in range(2):
                    o_ps = psum.tile([128, 512], F32, tag="sc", name="o_ps",
                                     bufs=3)
                    for i in range(NDC):
                        nc.tensor.matmul(
                            o_ps[:], vf[i][:],
                            wo_sb[i][:, 512 * dcb:512 * (dcb + 1)],
                            start=(i == 0), stop=(i == NDC - 1))
                    o_sb = stream.tile([128, 512], F32, tag="osb", bufs=2)
                    # copy + bias add fused (bo_eff pre-broadcast to 128 rows)
                    nc.vector.scalar_tensor_tensor(
                        o_sb[:], o_ps[:], 1.0,
                        bo_bc[:, 512 * dcb:512 * (dcb + 1)],
                        mybir.AluOpType.bypass, ADD)
                    for bb in range(B):
                        nc.scalar.dma_start(
                            out=out[PIECE * (B * blk + bb):
                                    PIECE * (B * blk + bb + 1),
                                    512 * dcb:512 * (dcb + 1)],
                            in_=o_sb[PIECE * bb:PIECE * (bb + 1), :])

            # outprojs all run after attn3: a straggler rank's late AllToAll
            # then never stalls the PE mid-attention (which would cascade the
            # skew); blocks 0-2 are guaranteed ready, and only outproj(3)
            # truly waits on the last collective (~4.5us exposure)
            attn_block(0)
            attn_block(1)
            attn_block(2, mid_hook=lambda: outproj_load(0))
            attn_block(3, mid_hook=lambda: (outproj_load(1),
                                            outproj_load(2)))
            for blk in range(NBLK - 1):
                outproj_block(blk)
            outproj_load(NBLK - 1, split=True)
            outproj_block(NBLK - 1)

    nc.compile()
    return nc


last_results = None


def kernel(x, y, mask, Wkv, bkv, Wq, bq, Wo, bo):
    x = np.asarray(x, dtype=np.float32)
    y = np.asarray(y, dtype=np.float32)
    Wkv = np.asarray(Wkv, dtype=np.float32)
    bkv = np.asarray(bkv, dtype=np.float32)
    Wq = np.asarray(Wq, dtype=np.float32)
    bq = np.asarray(bq, dtype=np.float32)
    Wo = np.asarray(Wo, dtype=np.float32)
    bo = np.asarray(bo, dtype=np.float32)

    wkv3 = Wkv.reshape(D, H, 2 * HD)
    bv = bkv.reshape(H, 2 * HD)[:, HD:].reshape(H * HD)
    bo_eff = (bv @ Wo + bo).astype(np.float32)

    nc = build_kernel()
    in_maps = []
    for c in range(N_CORES):
        b, j = divmod(c, GROUP)
        hs = HPC * j
        f16 = np.float16
        wq_sl = Wq[:, HD * hs:HD * (hs + HPC)]
        wk_sl = wkv3[:, hs:hs + HPC, :HD].reshape(D, NV)
        wv_sl = wkv3[:, hs:hs + HPC, HD:].reshape(D, NV)
        in_maps.append({
            "yT": np.ascontiguousarray(y[b].T).astype(f16),
            "xT": np.ascontiguousarray(x[b].T).astype(f16),
            "wqkv": np.ascontiguousarray(
                np.concatenate([wq_sl, wk_sl, wv_sl], axis=1)).astype(f16),
            "wo": Wo.astype(f16),
            "bq": np.ascontiguousarray(bq[HD * hs:HD * (hs + HPC)]),
            "bo": bo_eff.astype(f16),
        })

    import os
    import time
    time.sleep(0.5)  # settle DMA/thermal state from input upload
    trace = bool(os.environ.get("KERNEL_TRACE"))
    res = run_bass_kernel_spmd(nc, in_maps, core_ids=list(range(N_CORES)),
                               trace=trace)
    global last_results
    last_results = res

    full = np.empty((B, S, D), dtype=np.float32)
    for c in range(N_CORES):
        o = res.results[c]["out"].reshape(NBLK, B, PIECE, D)
        for blk in range(NBLK):
            for bb in range(B):
                s0 = SQB * blk + PIECE * c
                full[bb, s0:s0 + PIECE] = o[blk, bb]
    return full


# revision 32
# speedup vs baseline: 1.1474x; 1.1474x over previous
"""Multi-head cross-attention kernel for Trainium2, 8 NeuronCores.

Reference computation (B=2, S=2048, D=1024, H=16, hd=64):
    kv = x @ Wkv + bkv ; q = y @ Wq + bq
    per head: s = q k^T / 8 (+ mask, all-zero per spec), a = softmax(s)
    out = concat_h(a v) @ Wo + bo

Sharding: batch (2-way) x head-groups (4 heads/core).  Cores 0-3 own batch 0,
cores 4-7 own batch 1; within a batch group, core j owns heads 4j..4j+3 and,
after an AllToAll of normalized per-head attention outputs, computes the
output projection for four disjoint 64-row sq slices (one per 512-sq block).

Host-side shard prep:
  - x[b], y[b] transposed to [D, S] fp16 so the contraction dim lands on
    SBUF partitions.
  - Wq / Wk / Wv sliced per head group and packed into one [D, 768] tensor;
    k-bias dropped (softmax shift-invariant), v-bias folded into an
    effective output bias bo_eff = bv @ Wo + bo.
  - mask is all-zeros per the problem spec -> additive zero, skipped.

Device dataflow per core (its batch b, heads hg..hg+3), all matmuls fp16
with fp32 PSUM accumulation:
  kT[pair, S], qT[pair, S] = W^T xT / W^T yT;  v[S, 4x(64|ones)]
  per (sq-block 512, head pair): per sk-chunk 128:
    scoresT = row-packed K=64 matmul pair -> PSUM
    exp: head0 on ACT (exp, scale=1/8); head1 on DVE via Schraudolph
      bit-trick (i16 = s*(1477.32/8) + 15315.5, bitcast i16->f16 ~= 2^t,
      max rel err ~3%) -- splits softmax-exp across two engines so the
      PE stream, not ACT, paces the attention phase.
    PV accumulate with lhsT=[v|1] (M=65): unnormalized vals + denominator.
  normalize (DMA-free: engines only, so a straggler rank's pending
    collective -- whose descriptors gate shared DMA rings -- can never
    stall this chain or, transitively, the PE): d row -> fp16 SBUF row,
    K=1 matmul broadcasts it down 64 PSUM partitions,
    reciprocal_approx_fast off PSUM, one fused scalar_tensor_tensor
    (PSUM * rep -> fresh per-(blk,h) fp16 nv tile).
  Per-block 8-rank AllToAll of normalized valsT (fp16). vf tiles for the
  output projection are prefetched mid-attention on the idle sync queue;
  all four outprojs run after attn3 (only the last ~4.5us waits on the
  final collective). A small full-array matmul every 4th chunk keeps the
  HAM activity monitor from re-throttling the PE clock (attention
  matmuls alone use half the array and read as "idle").
  outproj: vf tiles pack both batches' rows (M=128), out = vf^T Wo + bo.
"""

import numpy as np

import concourse.bass as bass
import concourse.bacc as bacc
import concourse.mybir as mybir
from concourse.tile import TileContext
from concourse.bass_utils import run_bass_kernel_spmd

B, S, D = 2, 2048, 1024
H, HD = 16, 64
N_CORES = 8
GROUP = 4              # cores per batch group
HPC = H // GROUP       # heads per core (4)
NV = HPC * HD          # local vals rows (256)
NBLK = 4               # sq blocks
SQB = S // NBLK        # 512
PIECE = SQB // N_CORES  # 64: sq rows delivered to each rank per AllToAll
NKC = S // 128         # 16 sk chunks
NDC = D // 128         # 8 contraction chunks

F32 = mybir.dt.float32
FP16 = mybir.dt.float16
I16 = mybir.dt.int16
EXP = mybir.ActivationFunctionType.Exp
MULT = mybir.AluOpType.mult
ADD = mybir.AluOpType.add

SCALE = 0.125          # 1/sqrt(HD)
A16 = float(1024.0 / np.log(2.0)) * SCALE   # Schraudolph mult (scale folded)
B16 = float(1024 * 15 - 44.5)               # Schraudolph bias


def build_kernel():
    nc = bacc.Bacc("TRN2", target_bir_lowering=False, debug=False,
                   num_devices=N_CORES)

    yT = nc.declare_dram_parameter("yT", [D, S], FP16, isOutput=False)
    xT = nc.declare_dram_parameter("xT", [D, S], FP16, isOutput=False)
    # packed [q | k | v] head-group weight slices
    wqkv = nc.declare_dram_parameter("wqkv", [D, 3 * NV], FP16, isOutput=False)
    wo = nc.declare_dram_parameter("wo", [D, D], FP16, isOutput=False)
    bq = nc.declare_dram_parameter("bq", [NV], F32, isOutput=False)
    bo = nc.declare_dram_parameter("bo", [D], FP16, isOutput=False)
    # out rows: (blk, batch, 64 sq) for this rank's sq windows
    out = nc.declare_dram_parameter("out", [NBLK * B * PIECE, D], F32,
                                    isOutput=True)

    # 8-rank AllToAll: shard j = my heads' vals for rank j's sq window.
    cc_in = [nc.dram_tensor(f"cc_in{b}", [N_CORES, NV, PIECE], FP16)
             for b in range(NBLK)]
    cc_out = [nc.dram_tensor(f"cc_out{b}", [N_CORES * NV, PIECE], FP16)
              for b in range(NBLK)]
    groups = [[0, 1, 2, 3, 4, 5, 6, 7]]

    with TileContext(nc) as tc:
        with (
            tc.tile_pool(name="acts", bufs=1) as acts,        # persistent
            tc.tile_pool(name="stream", bufs=2) as stream,
            tc.tile_pool(name="attn", bufs=3) as attn,
            tc.tile_pool(name="psum", bufs=1, space="PSUM") as psum,
        ):
            # ---- persistent tiles ----
            xT_sb = [acts.tile([128, S], FP16, tag=f"xT{i}", name=f"xT{i}")
                     for i in range(NDC)]
            yT_sb = [acts.tile([128, S], FP16, tag=f"yT{i}", name=f"yT{i}")
                     for i in range(NDC)]
            wqkv_sb = [acts.tile([128, 3 * NV], FP16, tag=f"wqkv{i}",
                                 name=f"wqkv{i}") for i in range(NDC)]
            wo_sb = [acts.tile([128, D], FP16, tag=f"wo{i}", name=f"wo{i}")
                     for i in range(NDC)]
            qT_sb = [acts.tile([128, S], FP16, tag=f"qT{i}", name=f"qT{i}")
                     for i in range(2)]
            kT_sb = [acts.tile([128, S], FP16, tag=f"kT{i}", name=f"kT{i}")
                     for i in range(2)]
            v_sb = [acts.tile([128, HPC * (HD + 1)], FP16, tag=f"v{i}",
                              name=f"v{i}") for i in range(NKC)]
            ones_col = acts.tile([HD + 1, HD], FP16, tag="ones_col")
            bq_sb = acts.tile([128, 2], F32, tag="bq")
            bo_bc = acts.tile([128, D], FP16, tag="bo")

            nc.vector.memset(ones_col[:], 1.0)
            # load priority: weights for k/v proj, then x, then y, wo last.
            # dma_start costs ~600ns of sequencer issue time, so the 16
            # first-matmul-critical loads are split across both HWDGE
            # issuers (wqkv on ACT, xT on SP) to halve the preamble
            for i in range(NDC):
                nc.scalar.dma_start(out=wqkv_sb[i][:],
                                    in_=wqkv[128 * i:128 * (i + 1), :])
                nc.sync.dma_start(
                    out=xT_sb[i][:, 0:512],
                    in_=xT[128 * i:128 * (i + 1), 0:512])
            nc.scalar.dma_start(out=bq_sb[:],
                                in_=bq.rearrange("(c p) -> p c", p=128))
            nc.scalar.dma_start(out=bo_bc[:],
                                in_=bo[None, :].to_broadcast((128, D)))
            for sb in range(1, 4):
                for i in range(NDC):
                    nc.sync.dma_start(
                        out=xT_sb[i][:, 512 * sb:512 * (sb + 1)],
                        in_=xT[128 * i:128 * (i + 1), 512 * sb:512 * (sb + 1)])
            for sb in range(4):
                for i in range(NDC):
                    nc.sync.dma_start(
                        out=yT_sb[i][:, 512 * sb:512 * (sb + 1)],
                        in_=yT[128 * i:128 * (i + 1), 512 * sb:512 * (sb + 1)])
            for i in range(NDC):
                nc.sync.dma_start(out=wo_sb[i][:],
                                  in_=wo[128 * i:128 * (i + 1), :])

            # ---- projections ----
            # kT from xT
            for sb in range(4):
                for cc in range(2):
                    ps_k = psum.tile([128, 512], F32, tag="sc", name="ps_k",
                                     bufs=3)
                    for i in range(NDC):
                        nc.tensor.matmul(
                            ps_k[:],
                            wqkv_sb[i][:, NV + 128 * cc:NV + 128 * (cc + 1)],
                            xT_sb[i][:, 512 * sb:512 * (sb + 1)],
                            start=(i == 0), stop=(i == NDC - 1))
                    nc.vector.tensor_copy(
                        kT_sb[cc][:, 512 * sb:512 * (sb + 1)], ps_k[:])
            # v from xT
            for ks in range(NKC):
                ps_v = psum.tile([128, NV], F32, tag="pv", name="ps_v", bufs=4)
                for i in range(NDC):
                    nc.tensor.matmul(
                        ps_v[:], xT_sb[i][:, 128 * ks:128 * (ks + 1)],
                        wqkv_sb[i][:, 2 * NV:3 * NV],
                        start=(i == 0), stop=(i == NDC - 1))
                nc.vector.memset(v_sb[ks][:], 1.0)
                nc.vector.tensor_copy(
                    v_sb[ks][:].rearrange("p (h c) -> p h c",
                                          c=HD + 1)[:, :, 0:HD],
                    ps_v[:].rearrange("p (h c) -> p h c", c=HD))
            # qT from yT (with bias)
            for sb in range(4):
                for cc in range(2):
                    ps_q = psum.tile([128, 512], F32, tag="sc", name="ps_q",
                                     bufs=3)
                    for i in range(NDC):
                        nc.tensor.matmul(
                            ps_q[:],
                            wqkv_sb[i][:, 128 * cc:128 * (cc + 1)],
                            yT_sb[i][:, 512 * sb:512 * (sb + 1)],
                            start=(i == 0), stop=(i == NDC - 1))
                    nc.vector.tensor_scalar_add(
                        qT_sb[cc][:, 512 * sb:512 * (sb + 1)], ps_q[:],
                        bq_sb[:, cc:cc + 1])

            # ---- attention + pipelined outproj ----
            def normalize_pair(blk, pair, pv_ps):
                # DMA-free normalize: engines only, so a straggler rank's
                # pending collective (which gates descriptors in the shared
                # DMA rings) can never stall this chain or the PE stream
                sq0 = SQB * blk
                for hh in range(2):
                    h = 2 * pair + hh
                    # d row (partition 64) -> SBUF fp16 row, then a K=1
                    # matmul broadcasts it down 64 PSUM partitions
                    dstage = stream.tile([HD + 1, SQB], FP16, tag="dstage")
                    nc.scalar.copy(dstage[HD:HD + 1, :],
                                   pv_ps[hh][HD:HD + 1, :])
                    rep_ps = psum.tile([128, 512], F32, tag="op",
                                       name="rep_ps", bufs=1)
                    nc.tensor.matmul(rep_ps[0:HD, :],
                                     ones_col[HD:HD + 1, :],
                                     dstage[HD:HD + 1, :],
                                     tile_position=(64, 0))
                    rep = stream.tile([HD, SQB], F32, tag="rep")
                    nc.vector.reciprocal_approx_fast(rep[:], rep_ps[0:HD, :])
                    # fresh per-(blk,h) nv tile: a pending collective's
                    # cc_in read can then never WAR-block the next block's
                    # normalize (and transitively the PE)
                    nv = stream.tile([HD, SQB], FP16, tag=f"nv{h}",
                                     name=f"nv{h}", bufs=2)
                    nc.vector.scalar_tensor_tensor(
                        nv[:], pv_ps[hh][0:HD, :], 1.0, rep[:],
                        mybir.AluOpType.bypass, MULT)
                    nc.scalar.dma_start(
                        out=cc_in[blk][:, HD * h:HD * (h + 1), :]
                        .rearrange("g p q -> p g q"),
                        in_=nv[:])

            def attn_block(blk, mid_hook=None):
                # Software-pipelined chunk stream, flattened across the two
                # head pairs: scores(k+1) is emitted before PV(k) so the PE
                # never waits on exp; a tiny full-array matmul per chunk
                # keeps the HAM activity monitor from re-throttling the PE
                # clock (attention matmuls alone use half the array).
                sq0 = SQB * blk
                pv_tiles = {}
                inflight = []

                def emit_pv(pair, sc):
                    for hh in range(2):
                        h = 2 * pair + hh
                        nc.tensor.matmul(
                            pv_tiles[pair][hh][:],
                            v_sb[sc][:, (HD + 1) * h:(HD + 1) * (h + 1)],
                            at_tiles[(pair, sc)][hh][:],
                            start=(sc == 0), stop=(sc == NKC - 1))

                at_tiles = {}
                for k, (pair, sc) in enumerate(
                        (p, s) for p in range(2) for s in range(NKC)):
                    if sc == 0:
                        pv_tiles[pair] = [
                            psum.tile([128, SQB], F32, tag="pv",
                                      name=f"pv{hh}", bufs=4)[:HD + 1, :]
                            for hh in range(2)]
                    sc_ps = [psum.tile([128, SQB], F32, tag="sc",
                                       name=f"sc{hh}", bufs=3)
                             for hh in range(2)]
                    at_sb = [attn.tile([128, SQB], FP16, tag=f"at{hh}",
                                       name=f"at{hh}", bufs=4)
                             for hh in range(2)]
                    at_tiles[(pair, sc)] = at_sb
                    for hh in range(2):  # row-packed head pair
                        nc.tensor.matmul(
                            sc_ps[hh][:],
                            kT_sb[pair][64 * hh:64 * (hh + 1),
                                        128 * sc:128 * (sc + 1)],
                            qT_sb[pair][64 * hh:64 * (hh + 1),
                                        sq0:sq0 + SQB],
                            tile_position=(64 * hh, 0))
                    if k % 8 == 0:
                        # HAM warm-keeper: full-array K=128 M=128, N=32
                        warm_ps = psum.tile([128, 512], F32, tag="op",
                                            name="warm_ps", bufs=1)
                        nc.tensor.matmul(warm_ps[:, 0:32],
                                         wqkv_sb[0][:, 0:128],
                                         xT_sb[0][:, 0:32])
                    # exp split across engines: ACT for head0, DVE
                    # Schraudolph for head1
                    nc.scalar.activation(at_sb[0][:], sc_ps[0][:], EXP,
                                         scale=SCALE)
                    nc.vector.tensor_scalar(
                        at_sb[1][:].bitcast(I16), sc_ps[1][:],
                        A16, B16, MULT, ADD)
                    inflight.append((pair, sc))
                    if len(inflight) > 2:
                        done = inflight.pop(0)
                        emit_pv(*done)
                        if done[1] == NKC - 1:
                            normalize_pair(blk, done[0], pv_tiles[done[0]])
                        del at_tiles[done]
                    if k == NKC and mid_hook is not None:
                        mid_hook()
                for done in inflight:
                    emit_pv(*done)
                    if done[1] == NKC - 1:
                        normalize_pair(blk, done[0], pv_tiles[done[0]])
                    del at_tiles[done]
                nc.gpsimd.collective_compute(
                    "AllToAll", mybir.AluOpType.bypass,
                    ins=[cc_in[blk][:]], outs=[cc_out[blk][:]],
                    replica_groups=groups)

            vf_tiles = {}

            def outproj_load(blk, split=False):
                # vf tiles pack both batches: cols 0-63 batch0, 64-127
                # batch1 -- one 3D-AP DMA per tile, and (for the last,
                # tail-critical block) issue split across both HWDGE
                # queues to halve sequencer issue time
                vf = [stream.tile([128, 128], FP16, tag=f"vf{i}",
                                  name=f"vf{i}", bufs=4) for i in range(NDC)]
                vf_tiles[blk] = vf
                src3 = cc_out[blk].rearrange("(j p) q -> p j q", j=2)
                for i in range(NDC):
                    eng = nc.scalar if (split and i % 2) else nc.sync
                    eng.dma_start(
                        out=vf[i][:].rearrange("p (j q) -> p j q", j=2),
                        in_=src3[128 * i:128 * (i + 1)])

            def outproj_block(blk):
                vf = vf_tiles[blk]
                for dcb in range(2):
                    o_ps = psum.tile([128, 512], F32, tag="sc", name="o_ps",
                                     bufs=3)
                    for i in range(NDC):
                        nc.tensor.matmul(
                            o_ps[:], vf[i][:],
                            wo_sb[i][:, 512 * dcb:512 * (dcb + 1)],
                            start=(i == 0), stop=(i == NDC - 1))
                    o_sb = stream.tile([128, 512], F32, tag="osb", bufs=2)
                    # copy + bias add fused (bo_eff pre-broadcast to 128 rows)
                    nc.vector.scalar_tensor_tensor(
                        o_sb[:], o_ps[:], 1.0,
                        bo_bc[:, 512 * dcb:512 * (dcb + 1)],
                        mybir.AluOpType.bypass, ADD)
                    for bb in range(B):
                        nc.scalar.dma_start(
                            out=out[PIECE * (B * blk + bb):
                                    PIECE * (B * blk + bb + 1),
                                    512 * dcb:512 * (dcb + 1)],
                            in_=o_sb[PIECE * bb:PIECE * (bb + 1), :])

            # outprojs all run after attn3: a straggler rank's late AllToAll
            # then never stalls the PE mid-attention (which would cascade the
            # skew); blocks 0-2 are guaranteed ready, and only outproj(3)
            # truly waits on the last collective (~4.5us exposure)
            attn_block(0)
            attn_block(1)
            attn_block(2, mid_hook=lambda: outproj_load(0))
            attn_block(3, mid_hook=lambda: (outproj_load(1),
                                            outproj_load(2)))
            for blk in range(NBLK - 1):
                outproj_block(blk)
            outproj_load(NBLK - 1, split=True)
            outproj_block(NBLK - 1)

    nc.compile()
    return nc


last_results = None


def kernel(x, y, mask, Wkv, bkv, Wq, bq, Wo, bo):
    x = np.asarray(x, dtype=np.float32)
    y = np.asarray(y, dtype=np.float32)
    Wkv = np.asarray(Wkv, dtype=np.float32)
    bkv = np.asarray(bkv, dtype=np.float32)
    Wq = np.asarray(Wq, dtype=np.float32)
    bq = np.asarray(bq, dtype=np.float32)
    Wo = np.asarray(Wo, dtype=np.float32)
    bo = np.asarray(bo, dtype=np.float32)

    wkv3 = Wkv.reshape(D, H, 2 * HD)
    bv = bkv.reshape(H, 2 * HD)[:, HD:].reshape(H * HD)
    bo_eff = (bv @ Wo + bo).astype(np.float32)

    nc = build_kernel()
    in_maps = []
    for c in range(N_CORES):
        b, j = divmod(c, GROUP)
        hs = HPC * j
        f16 = np.float16
        wq_sl = Wq[:, HD * hs:HD * (hs + HPC)]
        wk_sl = wkv3[:, hs:hs + HPC, :HD].reshape(D, NV)
        wv_sl = wkv3[:, hs:hs + HPC, HD:].reshape(D, NV)
        in_maps.append({
            "yT": np.ascontiguousarray(y[b].T).astype(f16),
            "xT": np.ascontiguousarray(x[b].T).astype(f16),
            "wqkv": np.ascontiguousarray(
                np.concatenate([wq_sl, wk_sl, wv_sl], axis=1)).astype(f16),
            "wo": Wo.astype(f16),
            "bq": np.ascontiguousarray(bq[HD * hs:HD * (hs + HPC)]),
            "bo": bo_eff.astype(f16),
        })

    import os
    import time
    time.sleep(0.5)  # settle DMA/thermal state from input upload
    trace = bool(os.environ.get("KERNEL_TRACE"))
    res = run_bass_kernel_spmd(nc, in_maps, core_ids=list(range(N_CORES)),
                               trace=trace)
    global last_results
    last_results = res

    full = np.empty((B, S, D), dtype=np.float32)
    for c in range(N_CORES):
        o = res.results[c]["out"].reshape(NBLK, B, PIECE, D)
        for blk in range(NBLK):
            for bb in range(B):
                s0 = SQB * blk + PIECE * c
                full[bb, s0:s0 + PIECE] = o[blk, bb]
    return full
